# revision 26
# baseline (speedup 1.0000x reference)
"""Multi-head attention (B=2, N=2048, D=1024, H=16) on 8 trn2 NeuronCores.

Sharding: DP2 (batch) x TP4 (head quarters).  Core c handles batch c//4 and
heads [4*(c%4), 4*(c%4)+4).  Per core:
  - QKV projection for its 256 local dims (x^T streamed, weights stationary)
  - causal flash-style attention (no max subtraction: scores/32 are tiny so
    exp is safe; softmax denominator comes from a ones-column fused into the
    z-accumulation matmul)
  - 8-rank AllToAll of normalized z^T (bf16): each core sends, for every
    same-group peer, that peer's 512-query slice of its 128 local pair-dims
    (cross-group blocks are never read).  Two A2As (one per head pair) so the
    first overlaps the second pair's attention.
  - output projection for the core's 512-row slice of the sequence

The attention inner loop is ACT(exp)-bound while the projections are
PE-bound, so projection matmuls are interleaved into the attention emission
at a per-k-tile budget matching the ACT/PE time deficit; this keeps the PE
continuously busy (HAM stays un-throttled) and hides the projection phases
inside the attention wall-clock.

Host: slices/casts inputs, concatenates the 8 disjoint output slices.
"""

import os
import sys

for _p in ("/opt/trn_rl_repo", "/root/.axon_site/_ro/trn_rl_repo"):
    if os.path.isdir(_p) and _p not in sys.path:
        sys.path.append(_p)

import numpy as np
import ml_dtypes

import concourse.bass as bass
import concourse.mybir as mybir
import concourse.tile as tile
from concourse import bacc

B, N, D, H, HD = 2, 2048, 1024, 16, 64
NCORES, TP = 8, 4
DLOC = D // TP            # 256 local dims (4 heads) per core
P = 128
KT_X = D // P             # 8 contraction tiles for projections
NT = N // P               # 16 n-tiles
QB = 512                  # query block (PSUM bank width in fp32)
NQB = N // QB             # 4
NSLICE = N // TP          # 512 output rows per core
SCALE = 1.0 / 32.0        # 1/sqrt(D)

F32 = mybir.dt.float32
BF16 = mybir.dt.bfloat16
BF = ml_dtypes.bfloat16
Alu = mybir.AluOpType
Act = mybir.ActivationFunctionType


def build_bass():
    nc = bacc.Bacc("TRN2", num_devices=NCORES)

    xT = nc.dram_tensor("xT", [D, N], BF16, kind="ExternalInput")
    wq = nc.dram_tensor("wq", [D, DLOC], BF16, kind="ExternalInput")
    wk = nc.dram_tensor("wk", [D, DLOC], BF16, kind="ExternalInput")
    wv = nc.dram_tensor("wv", [D, DLOC], BF16, kind="ExternalInput")
    wo = nc.dram_tensor("wo", [D, D], BF16, kind="ExternalInput")
    bq = nc.dram_tensor("bq", [DLOC], F32, kind="ExternalInput")
    bk = nc.dram_tensor("bk", [DLOC], F32, kind="ExternalInput")
    bv = nc.dram_tensor("bv", [DLOC], F32, kind="ExternalInput")
    bo = nc.dram_tensor("bo", [D], F32, kind="ExternalInput")
    qoff = nc.dram_tensor("qoff", [1, 1], mybir.dt.uint32, kind="ExternalInput")
    out = nc.dram_tensor("out", [NSLICE, D], BF16, kind="ExternalOutput")

    with tile.TileContext(nc) as tc:
        with (
            tc.tile_pool(name="persist", bufs=1) as persist,
            tc.tile_pool(name="wtp", bufs=4) as wtp,
            tc.tile_pool(name="small", bufs=4) as small,
            tc.tile_pool(name="psA", bufs=2, space="PSUM") as psA,
            tc.tile_pool(name="psS", bufs=2, space="PSUM") as psS,
            tc.tile_pool(name="psZ", bufs=2, space="PSUM") as psZ,
            tc.tile_pool(name="dram", bufs=1, space="DRAM") as dram,
        ):
            from concourse.tile_rust import add_dep_helper as _adh

            # ---- input loads, spread across queues so xT streams at >1
            # queue's bandwidth and compute starts on the first k-tiles.
            # sync queue: the first-matmul critical path (wq + xT evens)
            # goes first; the 2 MB Wo load rides the sync tail where the
            # queue is otherwise idle until the post-collective DMAs.
            bq_sb = small.tile([P, 2], F32)
            nc.sync.dma_start(bq_sb, bq[:].rearrange("(t p) -> p t", p=P))
            # per-k-tile weight loads (contiguous 64 KB row blocks) paced
            # against the xT stream, so the projection's kt-inner matmuls
            # start on the first tiles instead of the last
            wq_sb = persist.tile([P, KT_X, DLOC], BF16)
            wk_sb = persist.tile([P, KT_X, DLOC], BF16)
            wv_sb = persist.tile([P, KT_X, DLOC], BF16)
            xT_sb = persist.tile([P, KT_X, N], BF16)
            for kt in range(KT_X):
                nc.sync.dma_start(wq_sb[:, kt], wq[kt * P:(kt + 1) * P, :])
                if kt % 2 == 0:
                    nc.sync.dma_start(xT_sb[:, kt], xT[kt * P:(kt + 1) * P, :])
            for kt in range(KT_X):
                nc.sync.dma_start(wk_sb[:, kt], wk[kt * P:(kt + 1) * P, :])
            bk_sb = persist.tile([P, 2], F32)
            nc.sync.dma_start(bk_sb, bk[:].rearrange("(t p) -> p t", p=P))
            bv_row = small.tile([1, DLOC], F32)
            nc.sync.dma_start(bv_row, bv[:].rearrange("(a d) -> a d", a=1))
            for kt in range(KT_X):
                nc.sync.dma_start(wv_sb[:, kt], wv[kt * P:(kt + 1) * P, :])
            bo_row = small.tile([1, D], F32)
            nc.sync.dma_start(bo_row, bo[:].rearrange("(a d) -> a d", a=1))
            qoff_sb = small.tile([1, 1], mybir.dt.uint32)
            nc.sync.dma_start(qoff_sb, qoff[:])
            wo_sb = persist.tile([P, KT_X, D], BF16)
            nc.sync.dma_start(wo_sb, wo[:].rearrange("(kt p) m -> p kt m", p=P))

            # scalar queue: odd xT tiles + a tiny exp to pull the ACT table
            # load off the attention critical path.
            nc.scalar.dma_start(xT_sb[:, 1], xT[P:2 * P, :])
            junk_sb = small.tile([P, 2], F32)
            nc.scalar.activation(junk_sb, bq_sb, Act.Exp)
            for kt in (3, 5, 7):
                nc.scalar.dma_start(xT_sb[:, kt], xT[kt * P:(kt + 1) * P, :])

            bqs_sb = persist.tile([P, 2], F32)
            nc.vector.tensor_scalar_mul(bqs_sb, bq_sb, SCALE)

            # gpsimd queue: mask, broadcasts, then the (late-needed) Wo load.
            # staircase causal mask: mask[kk, c] = 1 iff c >= kk + 384;
            # view [:, 384-o : 384-o+width] gives "keep iff qq >= kk + o"
            mask_sb = persist.tile([P, QB + 384], BF16)
            nc.gpsimd.memset(mask_sb, 1.0)
            nc.gpsimd.affine_select(
                out=mask_sb,
                in_=mask_sb,
                compare_op=Alu.is_ge,
                fill=0.0,
                base=-384,
                pattern=[[1, QB + 384]],
                channel_multiplier=-1,
            )
            bv_bc = persist.tile([P, DLOC], F32)
            nc.gpsimd.partition_broadcast(bv_bc, bv_row)
            bo_bc = persist.tile([P, D], F32)
            nc.gpsimd.partition_broadcast(bo_bc, bo_row)

            qT_sb = persist.tile([P, 2, N], BF16)
            kT_sb = persist.tile([P, 2, N], BF16)
            v_sb = persist.tile([P, NT, 4 * 65], BF16)
            nc.vector.memset(v_sb, 1.0)  # preset ones columns for denominators

            # ---- interleavable projection work units ----
            # Each generator yields an estimated PE cost (ns) per matmul it
            # emits; group-final element-wise evacuations cost no PE time.
            def qk_unit(mt, qc, which):
                ps = psA.tile([P, QB], F32, tag="proj", name=f"ps{which}")
                w_sb = wq_sb if which == "q" else wk_sb
                for kt in range(KT_X):
                    nc.tensor.matmul(
                        ps,
                        lhsT=w_sb[:, kt, mt * P:(mt + 1) * P],
                        rhs=xT_sb[:, kt, qc * QB:(qc + 1) * QB],
                        start=(kt == 0),
                        stop=(kt == KT_X - 1),
                    )
                    yield 340
                if which == "q":
                    nc.vector.tensor_scalar(
                        qT_sb[:, mt, qc * QB:(qc + 1) * QB],
                        ps,
                        SCALE,
                        bqs_sb[:, mt:mt + 1],
                        Alu.mult,
                        Alu.add,
                    )
                else:
                    nc.vector.tensor_scalar_add(
                        kT_sb[:, mt, qc * QB:(qc + 1) * QB],
                        ps,
                        bk_sb[:, mt:mt + 1],
                    )

            def v_unit(nt):
                psv_full = psA.tile([P, QB], F32, tag="proj", name="psv")
                psv = psv_full[:, :DLOC]
                for kt in range(KT_X):
                    nc.tensor.matmul(
                        psv,
                        lhsT=xT_sb[:, kt, nt * P:(nt + 1) * P],
                        rhs=wv_sb[:, kt, :],
                        start=(kt == 0),
                        stop=(kt == KT_X - 1),
                    )
                    yield 180
                nc.vector.tensor_tensor(
                    v_sb[:, nt].rearrange("p (h x) -> p h x", x=65)[:, :, 0:64],
                    psv.rearrange("p (h x) -> p h x", x=64),
                    bv_bc.rearrange("p (h x) -> p h x", x=64),
                    Alu.add,
                )

            class Feeder:
                def __init__(self):
                    self.units = []  # (marker, generator)

                def push(self, marker, gen):
                    self.units.append((marker, gen))

                def feed(self, budget_ns):
                    while budget_ns > 0 and self.units:
                        _, g = self.units[0]
                        try:
                            budget_ns -= next(g)
                        except StopIteration:
                            self.units.pop(0)

                def drain_until(self, markers):
                    """Emit every unit whose marker is in `markers` (units
                    are kept in order, so drain everything up to the last
                    such marker)."""
                    idx = -1
                    for i, (m, _) in enumerate(self.units):
                        if m in markers:
                            idx = i
                    if idx < 0:
                        return
                    for m, g in self.units[:idx + 1]:
                        for _ in g:
                            pass
                    del self.units[:idx + 1]

            feeder = Feeder()

            def run_all(gen):
                for _ in gen:
                    pass

            # per-core group block offset (g*4, g = batch group) for the A2A
            qregs = nc.alloc_registers()
            nc.regs_load(qregs, qoff_sb[0:1, 0:1])
            qoff_sv = nc.snap(qregs, donate=True)

            # one AllToAll per head pair so the first overlaps the second
            # pair's attention.  zin[pr] rows: block j = [my 128 pair-dims,
            # 512 queries of rank j]; zout[pr] block j = [rank j's 128
            # pair-dims, my 512 queries].
            zin = [dram.tile([NCORES * P, QB], BF16, name=f"zin{pr}")
                   for pr in range(2)]
            zout = [dram.tile([NCORES * P, QB], BF16, name=f"zout{pr}")
                    for pr in range(2)]

            # ---- attention for one head pair ----
            def attention(pr, qbs, need):
                for qb in qbs:
                    feeder.drain_until(need(qb))
                    kt_max = (qb + 1) * 4
                    zps = [psZ.tile([65, QB], F32, tag="z", name=f"zp{hi}")
                           for hi in range(2)]
                    # software-pipelined emission: the z matmuls of k-tile
                    # kt-2 are emitted after the score pair of k-tile kt, so
                    # the two score matmuls sit adjacent in the PE stream
                    # (running concurrently on disjoint row groups) and the
                    # exp of k-tile kt has two full iterations of PE work to
                    # complete before its z matmuls need it
                    pending_z = []
                    for kt in range(kt_max):
                        diag = kt >= qb * 4
                        o = kt * P - qb * QB if diag else 0
                        w = QB - o
                        # both heads' scores in one 2-bank PSUM tile so a
                        # single exp instruction covers the pair
                        spf = psS.tile([P, 2 * QB], F32, tag="score", name="spf")
                        s_insts = []
                        for hi in range(2):
                            sp = spf[:, hi * QB + o:(hi + 1) * QB]
                            si = nc.tensor.matmul(
                                sp,
                                lhsT=kT_sb[hi * 64:(hi + 1) * 64, pr,
                                           kt * P:(kt + 1) * P],
                                rhs=qT_sb[hi * 64:(hi + 1) * 64, pr,
                                          qb * QB + o:(qb + 1) * QB],
                                start=True,
                                stop=True,
                                tile_position=(hi * 64, 0),
                            )
                            s_insts.append(si)
                        ready = [a for it, a in pending_z if it <= kt - 2]
                        pending_z = [(it, a) for it, a in pending_z
                                     if it > kt - 2]
                        for args in ready:
                            zi = nc.tensor.matmul(**args)
                            # pin the static PE order to [S0,S1,z,...] so the
                            # two score matmuls stay adjacent and overlap on
                            # their disjoint row groups
                            _adh(zi.ins, s_insts[-1].ins, sync=False,
                                 reason="z after score pair")
                        wt = wtp.tile([P, 2 * QB], BF16, tag="wt", name="wt")
                        nc.scalar.activation(
                            wt[:].rearrange("p (h w) -> p h w", h=2)[:, :, o:QB],
                            spf[:].rearrange("p (h w) -> p h w", h=2)[:, :, o:QB],
                            Act.Exp,
                        )
                        for hi in range(2):
                            if diag:
                                # only the o..o+128 strip straddles the diagonal
                                nc.vector.tensor_tensor(
                                    wt[:, hi * QB + o:hi * QB + o + P],
                                    wt[:, hi * QB + o:hi * QB + o + P],
                                    mask_sb[:, 384:384 + P],
                                    Alu.mult,
                                )
                            pending_z.append((kt, dict(
                                out=zps[hi][:, o:QB],
                                lhsT=v_sb[:, kt,
                                          (2 * pr + hi) * 65:(2 * pr + hi + 1) * 65],
                                rhs=wt[:, hi * QB + o:(hi + 1) * QB],
                                start=(kt == 0),
                                stop=(kt == kt_max - 1),
                                skip_group_check=True,
                            )))
                        # top up the PE with projection matmuls to cover the
                        # ACT-vs-PE deficit of this k-tile (PE at the 13/16
                        # GPIO-throttled 1.95 GHz this box sustains)
                        feeder.feed((2 * w + 352) / 1.2 - 3 * w / 1.95)
                    for _, args in pending_z:
                        nc.tensor.matmul(**args)
                    for hi in range(2):
                        # denominator -> SBUF (approx_fast misbehaves on a
                        # PSUM source), reciprocal, gpsimd partition
                        # broadcast, normalize.  No PE in this chain; the
                        # hi=1 copy rides ScalarE so both heads' chains
                        # overlap and the collective triggers sooner.
                        den = small.tile([1, QB], F32, tag="den", name="den")
                        if hi == 0:
                            nc.vector.tensor_copy(den, zps[hi][64:65, :])
                        else:
                            nc.scalar.copy(den, zps[hi][64:65, :])
                        recip = small.tile([1, QB], F32, tag="recip", name="recip")
                        nc.vector.reciprocal_approx_fast(recip, den)
                        rb = small.tile([64, QB], F32, tag="rb", name="rb")
                        nc.gpsimd.partition_broadcast(rb, recip)
                        zn = small.tile([64, QB], BF16, tag="zn", name="zn")
                        nc.vector.tensor_tensor(zn, zps[hi][0:64, :], rb, Alu.mult)
                        nc.gpsimd.dma_start(
                            zin[pr][bass.ds(qoff_sv * P + (qb * P + hi * 64), 64),
                                    :],
                            zn,
                        )

            def gather(pr):
                return nc.gpsimd.collective_compute(
                    "AllToAll",
                    Alu.bypass,
                    replica_groups=[[0, 1, 2, 3, 4, 5, 6, 7]],
                    ins=[zin[pr][:].opt()],
                    outs=[zout[pr][:].opt()],
                )

            # ---- emission schedule ----
            # pre-attention: only what attn0 qb=0 needs; everything else is
            # interleaved into the attention stream by the feeder.
            run_all(qk_unit(0, 0, "q"))
            run_all(qk_unit(0, 0, "k"))
            for nt in range(4):
                run_all(v_unit(nt))

            feeder.push(("qk0", 1), qk_unit(0, 1, "q"))
            feeder.push(("qk0", 1), qk_unit(0, 1, "k"))
            for nt in range(4, 8):
                feeder.push(("v", nt), v_unit(nt))
            feeder.push(("qk0", 2), qk_unit(0, 2, "q"))
            feeder.push(("qk0", 2), qk_unit(0, 2, "k"))
            for nt in range(8, 12):
                feeder.push(("v", nt), v_unit(nt))
            feeder.push(("qk0", 3), qk_unit(0, 3, "q"))
            feeder.push(("qk0", 3), qk_unit(0, 3, "k"))
            for nt in range(12, 16):
                feeder.push(("v", nt), v_unit(nt))
            for qc in range(NQB):
                feeder.push(("qk1", qc), qk_unit(1, qc, "q"))
                feeder.push(("qk1", qc), qk_unit(1, qc, "k"))

            def need0(qb):
                ms = {("qk0", qc) for qc in range(qb + 1)}
                ms |= {("v", nt) for nt in range(4 * (qb + 1))}
                return ms

            def need1(qb):
                return {("qk1", qc) for qc in range(qb + 1)} | need0(3)

            attention(0, [0, 1, 2, 3], need0)
            gather(0)
            attention(1, [0, 1, 2, 3], need1)
            g1 = gather(1)
            # anything the feeder still holds runs here, during the A2A
            for _, g in feeder.units:
                for _ in g:
                    pass
            feeder.units.clear()

            # ---- output projection for this core's 512-row slice ----
            # Split by k-tile parity: even k-tiles only need the pair-0
            # AllToAll, so that half runs while the pair-1 collective is
            # still in flight; the odd half + combine follows it.
            zg_sb = persist.tile([P, KT_X, QB], BF16)
            stage_sb = persist.tile([P, NSLICE // P, D // QB, QB], F32)
            from concourse.bass import ds
            from concourse.tile_rust import add_dep_helper
            # even k-tiles (pair-0 A2A) first so the A2A#2-gated odd DMAs
            # don't block them on the in-order sync queue; one DMA per parity
            for par in (0, 1):
                zgd = nc.sync.dma_start(
                    zg_sb[:, par::2],
                    zout[par].rearrange("(j p) n -> p j n", p=P)[
                        :, ds(qoff_sv, TP), :],
                )
                # scheduling-order-only edge: keep these A2A-gated DMAs from
                # being placed ahead of attention(1) in the static order,
                # which would serialize attention behind the collective via
                # shared DMA-semaphore counts
                add_dep_helper(zgd.ins, g1.ins, sync=False,
                               reason="zg after gather(1) trigger")
            for mt in range(NSLICE // P):
                for oc in range(D // QB):
                    pse = psA.tile([P, QB], F32, tag="proj", name="pse")
                    for i, kt in enumerate(range(0, KT_X, 2)):
                        nc.tensor.matmul(
                            pse,
                            lhsT=zg_sb[:, kt, mt * P:(mt + 1) * P],
                            rhs=wo_sb[:, kt, oc * QB:(oc + 1) * QB],
                            start=(i == 0),
                            stop=(kt == KT_X - 2),
                        )
                    # fold the output bias in here, off the critical tail
                    nc.vector.tensor_tensor(
                        stage_sb[:, mt, oc], pse,
                        bo_bc[:, oc * QB:(oc + 1) * QB], Alu.add
                    )
            # warm-keeper: a DVE-paced chain of discarded matmuls that keeps
            # the PE active through the pair-1 collective window so the odd
            # half doesn't start on a HAM-throttled (half-clock) array.  The
            # chain is rooted on the zg DMA (so it can't run early) and each
            # matmul is gated by an in-place DVE op (~0.7us), spreading ~22
            # matmuls across the collective's in-flight time.
            pace_sb = persist.tile([P, QB], BF16)
            nc.vector.memset(pace_sb, 1.0)
            for i in range(8):
                nc.vector.tensor_tensor(
                    pace_sb, pace_sb, zg_sb[:, 0, 0:QB], Alu.mult
                )
                jnk = psA.tile([P, QB], F32, tag="proj", name="jnk")
                nc.tensor.matmul(
                    jnk,
                    lhsT=pace_sb[:, 0:P],
                    rhs=pace_sb,
                    start=True,
                    stop=False,
                )
                nc.tensor.matmul(
                    jnk,
                    lhsT=pace_sb[:, P:2 * P],
                    rhs=pace_sb,
                    start=False,
                    stop=True,
                )
            for mt in range(NSLICE // P):
                for oc in range(D // QB):
                    pso = psA.tile([P, QB], F32, tag="proj", name="pso")
                    for i, kt in enumerate(range(1, KT_X, 2)):
                        nc.tensor.matmul(
                            pso,
                            lhsT=zg_sb[:, kt, mt * P:(mt + 1) * P],
                            rhs=wo_sb[:, kt, oc * QB:(oc + 1) * QB],
                            start=(i == 0),
                            stop=(kt == KT_X - 1),
                        )
                    osb = small.tile([P, QB], BF16, tag="osb", name="osb")
                    nc.vector.tensor_tensor(
                        osb, pso, stage_sb[:, mt, oc], Alu.add
                    )
                    nc.sync.dma_start(
                        out[mt * P:(mt + 1) * P, oc * QB:(oc + 1) * QB], osb
                    )
    nc.compile()
    return nc


def make_in_maps(inputs):
    x = np.asarray(inputs["inputs"], dtype=np.float32)
    ws = {k: np.asarray(inputs[k], dtype=np.float32) for k in
          ("Wq", "Wk", "Wv", "Wo", "bq", "bk", "bv", "bo")}
    wo_bf = np.ascontiguousarray(ws["Wo"]).astype(BF)
    xT_bf = [np.ascontiguousarray(x[b].T).astype(BF) for b in range(B)]
    in_maps = []
    for c in range(NCORES):
        b, q = c // TP, c % TP
        cols = slice(q * DLOC, (q + 1) * DLOC)
        in_maps.append({
            "xT": xT_bf[b],
            "wq": np.ascontiguousarray(ws["Wq"][:, cols]).astype(BF),
            "wk": np.ascontiguousarray(ws["Wk"][:, cols]).astype(BF),
            "wv": np.ascontiguousarray(ws["Wv"][:, cols]).astype(BF),
            "wo": wo_bf,
            "bq": np.ascontiguousarray(ws["bq"][cols]),
            "bk": np.ascontiguousarray(ws["bk"][cols]),
            "bv": np.ascontiguousarray(ws["bv"][cols]),
            "bo": ws["bo"],
            "qoff": np.array([[b * TP]], dtype=np.uint32),
        })
    return in_maps


def assemble(results):
    outs = [np.asarray(r["out"], dtype=np.float32) for r in results]
    return np.stack(
        [np.concatenate(outs[b * TP:(b + 1) * TP], axis=0) for b in range(B)]
    )


def _ensure_ntff_hook():
    """bass_utils hard-imports antenv.axon_hooks for trace=True; this image
    lacks it.  Shim it and register the ctypes NTFF hook from trn_boot."""
    import types

    if "antenv.axon_hooks" in sys.modules:
        return
    try:
        import antenv.axon_hooks  # noqa: F401
        return
    except ImportError:
        pass
    mod = types.ModuleType("antenv.axon_hooks")
    mod._hook = None
    mod.set_axon_ntff_profile_hook = lambda h: setattr(mod, "_hook", h)
    mod.get_axon_ntff_profile_hook = lambda: mod._hook
    sys.modules["antenv.axon_hooks"] = mod
    try:
        import antenv
        antenv.axon_hooks = mod
    except Exception:
        pass
    try:
        from trn_agent_boot.trn_boot import _ntff_profile_via_ctypes
        hook = _ntff_profile_via_ctypes("/opt/axon/libaxon_pjrt.so")
        if hook is not None:
            mod._hook = hook
    except Exception:
        pass


_cached_nc = None


def kernel(**inputs):
    global _cached_nc
    _ensure_ntff_hook()
    from concourse.bass_utils import run_bass_kernel_spmd

    if _cached_nc is None:
        _cached_nc = build_bass()
    trace = bool(int(os.environ.get("MHA_TRACE", "0")))
    res = run_bass_kernel_spmd(
        _cached_nc, make_in_maps(inputs), core_ids=list(range(NCORES)),
        trace=trace,
    )
    if trace and res.exec_time_ns is not None:
        print(f"HW exec time: {res.exec_time_ns} ns")
        kernel.last_exec_time_ns = res.exec_time_ns
    return assemble(res.results)
